# revision 48
# baseline (speedup 1.0000x reference)
"""Bass/Trainium2 kernel for FLAOperator(mode='gla') CPU-fallback scan.

Reference recurrence (per b, h, d lane, over t = 0..N-1):
    s_t = s_{t-1} + sigmoid(q_t * k_t + g_t) * v_t ;  y_t = s_t
i.e. y = cumsum over N of u, with u = sigmoid(q*k + g) * v  (pure elementwise).

Shapes: q,k,v,g,y all [B=2, H=16, N=4096, D=128] f32.

Strategy (8 NeuronCores, SPMD, no collectives).  Per core: 4 of the 32
independent (b,h) recurrences, processed as units over the time axis.
The kernel is DVE-bound (~6.6us per 2048-step unit), so every choice
below is about keeping the Vector engine saturated with a minimal op
count (measured baseline 92us -> this kernel ~77us):

  - Host prep (layout only): per (b,h) slab transpose to [D, N], split
    into 2 time chunks, de-interleave each into even/odd halves.
  - Dtypes: q,k,g are fp8(E3M4) in HBM, upcast to bf16 by the SWDGE
    (gpsimd) cast-DMA during the load, so DVE ops stay in the fast 2x
    packed-bf16 mode while input traffic drops 16->10 MiB/core (~0.6%
    relative error, gate is 2e-2).  v and y stay bf16 (their rounding
    feeds the output directly; fp8 there would blow the error budget).
  - Queue balance (measured, load-bearing): q,k,g casts on the single
    SWDGE ring, v loads on sync/HWDGE, y stores on scalar/HWDGE.  The
    SDMA engines round-robin BETWEEN rings, so flooding the two HWDGE
    rings with prefetch starves the SWDGE ring the compute actually
    waits on - v/g/y stay paced by the pipeline instead.
  - Ramp: the SWDGE path takes ~15us to deliver its first transfer
    (Q7 descriptor-generation warmup), so the first unit is split into
    two half-units loaded raw-fp8 over the low-latency sync/HWDGE path;
    DVE eats 1x-mode ops on those but starts ~9us earlier.  The last
    unit is also split to shorten the serial scan+store tail.
  - Radix-2 scan: DVE tensor_tensor_scan has TWO data operands
    (state = (data0 op0 state) op1 data1), so scan(uE, uO, add, add)
    yields the odd-position cumsum in HALF the columns (the serial scan
    costs ~2 cycles/column vs ~0.5 for TT ops).  Even positions are one
    subtract: yE = yO - uO.  Both run in place inside the u tile.
  - Elementwise work per unit on whole [128, 2*w] tiles: a = q*k (DVE),
    a += g (DVE), sigmoid (ACT), u = s*v (DVE in place).
  - Software pipeline with 1-unit lookahead so the ACT round-trip hides
    under the next unit's DVE muls; chunks of a (b,h) chain the scan via
    initial = prev yO's last column.  a-pool depth 6 so stage-1 muls
    never wait on earlier units' store completions.
"""

from contextlib import ExitStack

import ml_dtypes
import numpy as np

import concourse.bass as bass
import concourse.tile as tile
from concourse import bacc, mybir
from concourse.bass_utils import run_bass_kernel_spmd

B, H, N, D = 2, 16, 4096, 128
N_CORES = 8
BH = B * H                    # 32 independent recurrences
BH_PER_CORE = BH // N_CORES   # 4
P = 128                       # partitions (= D)
NCH = 2                       # time chunks per (b,h)
NT = N // NCH                 # time steps per chunk (2048)
N4 = NT // 2                  # columns per parity half (1024)
F32 = mybir.dt.float32
BF16 = mybir.dt.bfloat16
BF16_NP = ml_dtypes.bfloat16
F8E3 = mybir.dt.float8e3
F8E3_NP = ml_dtypes.float8_e3m4

LOOKAHEAD = 1

_PROGRAM = None       # cached compiled Bass program (module-level)
LAST_RESULTS = None   # BassKernelResults of the last run (for test harness)


def _build_program() -> bass.Bass:
    nc = bacc.Bacc("TRN2", debug=False, num_devices=N_CORES)

    shape = [BH_PER_CORE, NCH, 2, D, N4]
    q_d = nc.dram_tensor("q", shape, F8E3, kind="ExternalInput").ap()
    k_d = nc.dram_tensor("k", shape, F8E3, kind="ExternalInput").ap()
    v_d = nc.dram_tensor("v", shape, BF16, kind="ExternalInput").ap()
    g_d = nc.dram_tensor("g", shape, F8E3, kind="ExternalInput").ap()
    # y: [bh, chunk, half, d, n] where half 0 = odd-position results (yO),
    # half 1 = even-position results (yE) - matches the SBUF tile layout so
    # the store is one DMA; the host swaps the halves back.
    y_d = nc.dram_tensor("y", shape, BF16, kind="ExternalOutput").ap()

    # Units are (bh, chunk, col0, ncols) over the N4 pair-columns of a
    # chunk.  The first unit is split in half (and loaded raw over the
    # low-latency HWDGE path) so DVE has work while the SWDGE cast path
    # warms up; the last is split so the serial scan+sub+store tail after
    # the final load is shorter.
    H4 = N4 // 2
    units = [(0, 0, 0, H4), (0, 0, H4, H4)]
    units += [
        (bh, c, 0, N4)
        for bh in range(BH_PER_CORE)
        for c in range(NCH)
        if (bh, c) not in ((0, 0), (BH_PER_CORE - 1, NCH - 1))
    ]
    units += [(BH_PER_CORE - 1, NCH - 1, 0, H4), (BH_PER_CORE - 1, NCH - 1, H4, H4)]
    RAW_UNITS = 2
    NU = len(units)  # 10

    with tile.TileContext(nc) as tc, ExitStack() as ctx:
        const_pool = ctx.enter_context(tc.tile_pool(name="const", bufs=1))
        qkv_pool = ctx.enter_context(tc.tile_pool(name="qkv", bufs=NU))
        g_pool = ctx.enter_context(tc.tile_pool(name="g", bufs=NU))
        a_pool = ctx.enter_context(tc.tile_pool(name="a", bufs=6))

        # Dummy sigmoid so the ACT function table loads during the
        # framework preamble instead of stalling the first real unit.
        warm = const_pool.tile([P, 2], BF16, tag="warm")
        nc.vector.memset(warm[:], 0.0)
        nc.scalar.activation(warm[:], warm[:], mybir.ActivationFunctionType.Sigmoid)


        stage1 = {}   # unit -> (at, vt) awaiting stage 2
        prev_y = {}   # bh -> y tile of previous chunk (scan carry chain)

        def emit_stage1(u, raw=False):
            bh, c, o, w = u
            sl = slice(o, o + w)
            in_dt = F8E3 if raw else BF16
            eng = nc.sync if raw else nc.gpsimd
            qt = qkv_pool.tile([P, 2 * w], in_dt, tag="q")
            kt = qkv_pool.tile([P, 2 * w], in_dt, tag="k")
            gt = g_pool.tile([P, 2 * w], in_dt, tag="g")
            vt = qkv_pool.tile([P, 2 * w], BF16, tag="v")
            eng.dma_start(out=qt[:], in_=q_d[bh, c, :, :, sl].transpose([1, 0, 2]))
            eng.dma_start(out=kt[:], in_=k_d[bh, c, :, :, sl].transpose([1, 0, 2]))
            eng.dma_start(out=gt[:], in_=g_d[bh, c, :, :, sl].transpose([1, 0, 2]))
            nc.sync.dma_start(out=vt[:], in_=v_d[bh, c, :, :, sl].transpose([1, 0, 2]))
            at = a_pool.tile([P, 2 * w], BF16, tag="a")
            nc.vector.tensor_mul(at[:], qt[:], kt[:])          # a = q*k
            nc.vector.tensor_add(at[:], at[:], gt[:])          # a += g
            nc.scalar.activation(
                at[:], at[:], mybir.ActivationFunctionType.Sigmoid
            )
            stage1[u] = (at, vt)

        def emit_stage2(u):
            bh, c, o, w = u
            sl = slice(o, o + w)
            at, vt = stage1.pop(u)
            nc.vector.tensor_mul(at[:], at[:], vt[:])          # u = s*v
            # yO = cumsum of (uE + uO) pairs: radix-2 scan over w columns,
            # written in place over uE (column t is read before written).
            if (c, o) != (0, 0):
                pt, pw = prev_y[bh]
                init = pt[:, pw - 1 : pw]
            else:
                init = 0.0
            nc.vector.tensor_tensor_scan(
                out=at[:, :w], data0=at[:, :w], data1=at[:, w:],
                initial=init,
                op0=mybir.AluOpType.add, op1=mybir.AluOpType.add,
            )
            nc.scalar.dma_start(out=y_d[bh, c, 1, :, sl], in_=at[:, :w])  # yO
            # yE = yO - uO, in place over uO.
            nc.vector.tensor_sub(at[:, w:], at[:, :w], at[:, w:])
            if u == units[-1]:
                # The kernel's end is gated by this store's transfer +
                # HBM write-receipt: split it across both HWDGE rings
                # (idle by now) so the two receipts overlap.
                h = w // 2
                nc.sync.dma_start(
                    out=y_d[bh, c, 0, :, o : o + h], in_=at[:, w : w + h]
                )
                nc.scalar.dma_start(
                    out=y_d[bh, c, 0, :, o + h : o + w], in_=at[:, w + h :]
                )
            else:
                nc.scalar.dma_start(out=y_d[bh, c, 0, :, sl], in_=at[:, w:])  # yE
            prev_y[bh] = (at, w)

        for i, u in enumerate(units):
            emit_stage1(u, raw=(i < RAW_UNITS))
            if i >= LOOKAHEAD:
                emit_stage2(units[i - LOOKAHEAD])
        for u in units[-LOOKAHEAD:]:
            emit_stage2(u)

    nc.compile()  # bacc backend: wait legalization, reg alloc, nop fusion
    return nc


def kernel(q: np.ndarray, k: np.ndarray, v: np.ndarray, g: np.ndarray) -> np.ndarray:
    global _PROGRAM, LAST_RESULTS
    if _PROGRAM is None:
        _PROGRAM = _build_program()

    def prep(x, dt):
        # [B,H,N,D] f32 -> [BH, NCH, 2, D, N4]: per (b,h), time-major per
        # d lane, chunked then de-interleaved into even/odd steps.
        x = np.asarray(x, dtype=np.float32).reshape(BH, NCH, N4, 2, D)
        return np.ascontiguousarray(x.transpose(0, 1, 3, 4, 2)).astype(dt)

    qp, kp = prep(q, F8E3_NP), prep(k, F8E3_NP)
    vp, gp = prep(v, BF16_NP), prep(g, F8E3_NP)
    in_maps = []
    for i in range(N_CORES):
        s = slice(i * BH_PER_CORE, (i + 1) * BH_PER_CORE)
        in_maps.append({"q": qp[s], "k": kp[s], "v": vp[s], "g": gp[s]})

    LAST_RESULTS = run_bass_kernel_spmd(_PROGRAM, in_maps, core_ids=list(range(N_CORES)))
    y = np.concatenate([r["y"] for r in LAST_RESULTS.results], axis=0)
    # y: [BH, NCH, 2, D, N4], parity 0 = even positions, 1 = odd.
    # -> [BH, NCH, N4, 2(par), D] -> [B, H, N, D]
    y = y.transpose(0, 1, 4, 2, 3)
    return np.ascontiguousarray(y).astype(np.float32).reshape(B, H, N, D)


# revision 49
# speedup vs baseline: 1.1475x; 1.1475x over previous
"""Bass/Trainium2 kernel for FLAOperator(mode='gla') CPU-fallback scan.

Reference recurrence (per b, h, d lane, over t = 0..N-1):
    s_t = s_{t-1} + sigmoid(q_t * k_t + g_t) * v_t ;  y_t = s_t
i.e. y = cumsum over N of u, with u = sigmoid(q*k + g) * v  (pure elementwise).

Shapes: q,k,v,g,y all [B=2, H=16, N=4096, D=128] f32.

Strategy (8 NeuronCores, SPMD, no collectives).  Per core: 4 of the 32
independent (b,h) recurrences, processed as units over the time axis.
The kernel is DVE-bound (~6.6us per 2048-step unit), so every choice
below is about keeping the Vector engine saturated with a minimal op
count (measured baseline 92us -> this kernel ~77us):

  - Host prep (layout only): per (b,h) slab transpose to [D, N], split
    into 2 time chunks, de-interleave each into even/odd halves.
  - Dtypes: q,k,g are fp8(E3M4) in HBM, upcast to bf16 by the SWDGE
    (gpsimd) cast-DMA during the load, so DVE ops stay in the fast 2x
    packed-bf16 mode while input traffic drops 16->10 MiB/core (~0.6%
    relative error, gate is 2e-2).  v and y stay bf16 (their rounding
    feeds the output directly; fp8 there would blow the error budget).
  - Queue balance (measured, load-bearing): q,k,g casts on the single
    SWDGE ring, v loads on sync/HWDGE, y stores on scalar/HWDGE.  The
    SDMA engines round-robin BETWEEN rings, so flooding the two HWDGE
    rings with prefetch starves the SWDGE ring the compute actually
    waits on - v/g/y stay paced by the pipeline instead.
  - Ramp: the SWDGE path takes ~15us to deliver its first transfer
    (Q7 descriptor-generation warmup), so the first unit is split into
    two half-units loaded raw-fp8 over the low-latency sync/HWDGE path;
    DVE eats 1x-mode ops on those but starts ~9us earlier.  The last
    unit is also split to shorten the serial scan+store tail.
  - Radix-2 scan: DVE tensor_tensor_scan has TWO data operands
    (state = (data0 op0 state) op1 data1), so scan(uE, uO, add, add)
    yields the odd-position cumsum in HALF the columns (the serial scan
    costs ~2 cycles/column vs ~0.5 for TT ops).  Even positions are one
    subtract: yE = yO - uO.  Both run in place inside the u tile.
  - Elementwise work per unit on whole [128, 2*w] tiles: a = q*k (DVE),
    a += g (DVE), sigmoid (ACT), u = s*v (DVE in place).
  - Software pipeline with 1-unit lookahead so the ACT round-trip hides
    under the next unit's DVE muls; chunks of a (b,h) chain the scan via
    initial = prev yO's last column.  a-pool depth 6 so stage-1 muls
    never wait on earlier units' store completions.
"""

from contextlib import ExitStack

import ml_dtypes
import numpy as np

import concourse.bass as bass
import concourse.tile as tile
from concourse import bacc, mybir
from concourse.bass_utils import run_bass_kernel_spmd

B, H, N, D = 2, 16, 4096, 128
N_CORES = 8
BH = B * H                    # 32 independent recurrences
BH_PER_CORE = BH // N_CORES   # 4
P = 128                       # partitions (= D)
NCH = 2                       # time chunks per (b,h)
NT = N // NCH                 # time steps per chunk (2048)
N4 = NT // 2                  # columns per parity half (1024)
F32 = mybir.dt.float32
BF16 = mybir.dt.bfloat16
BF16_NP = ml_dtypes.bfloat16
F8E3 = mybir.dt.float8e3
F8E3_NP = ml_dtypes.float8_e3m4

LOOKAHEAD = 1

_PROGRAM = None       # cached compiled Bass program (module-level)
LAST_RESULTS = None   # BassKernelResults of the last run (for test harness)


def _build_program() -> bass.Bass:
    nc = bacc.Bacc("TRN2", debug=False, num_devices=N_CORES)

    shape = [BH_PER_CORE, NCH, 2, D, N4]
    q_d = nc.dram_tensor("q", shape, F8E3, kind="ExternalInput").ap()
    k_d = nc.dram_tensor("k", shape, F8E3, kind="ExternalInput").ap()
    v_d = nc.dram_tensor("v", shape, BF16, kind="ExternalInput").ap()
    g_d = nc.dram_tensor("g", shape, F8E3, kind="ExternalInput").ap()
    # y: [bh, chunk, half, d, n] where half 0 = odd-position results (yO),
    # half 1 = even-position results (yE) - matches the SBUF tile layout so
    # the store is one DMA; the host swaps the halves back.
    y_d = nc.dram_tensor("y", shape, BF16, kind="ExternalOutput").ap()

    # Units are (bh, chunk, col0, ncols) over the N4 pair-columns of a
    # chunk.  The first unit is split in half (and loaded raw over the
    # low-latency HWDGE path) so DVE has work while the SWDGE cast path
    # warms up; the last is split so the serial scan+sub+store tail after
    # the final load is shorter.
    H4 = N4 // 2
    units = [(0, 0, 0, H4), (0, 0, H4, H4)]
    units += [
        (bh, c, 0, N4)
        for bh in range(BH_PER_CORE)
        for c in range(NCH)
        if (bh, c) not in ((0, 0), (BH_PER_CORE - 1, NCH - 1))
    ]
    units += [(BH_PER_CORE - 1, NCH - 1, 0, H4), (BH_PER_CORE - 1, NCH - 1, H4, H4)]
    RAW_UNITS = 2
    NU = len(units)  # 10

    with tile.TileContext(nc) as tc, ExitStack() as ctx:
        const_pool = ctx.enter_context(tc.tile_pool(name="const", bufs=1))
        qkv_pool = ctx.enter_context(tc.tile_pool(name="qkv", bufs=NU))
        g_pool = ctx.enter_context(tc.tile_pool(name="g", bufs=NU))
        a_pool = ctx.enter_context(tc.tile_pool(name="a", bufs=6))

        # Dummy sigmoid so the ACT function table loads during the
        # framework preamble instead of stalling the first real unit.
        warm = const_pool.tile([P, 2], BF16, tag="warm")
        nc.vector.memset(warm[:], 0.0)
        nc.scalar.activation(warm[:], warm[:], mybir.ActivationFunctionType.Sigmoid)


        stage1 = {}   # unit -> (at, vt) awaiting stage 2
        prev_y = {}   # bh -> y tile of previous chunk (scan carry chain)

        def emit_stage1(u, raw=False):
            bh, c, o, w = u
            sl = slice(o, o + w)
            in_dt = F8E3 if raw else BF16
            eng = nc.sync if raw else nc.gpsimd
            qt = qkv_pool.tile([P, 2 * w], in_dt, tag="q")
            kt = qkv_pool.tile([P, 2 * w], in_dt, tag="k")
            gt = g_pool.tile([P, 2 * w], in_dt, tag="g")
            vt = qkv_pool.tile([P, 2 * w], BF16, tag="v")
            eng.dma_start(out=qt[:], in_=q_d[bh, c, :, :, sl].transpose([1, 0, 2]))
            eng.dma_start(out=kt[:], in_=k_d[bh, c, :, :, sl].transpose([1, 0, 2]))
            eng.dma_start(out=gt[:], in_=g_d[bh, c, :, :, sl].transpose([1, 0, 2]))
            nc.sync.dma_start(out=vt[:], in_=v_d[bh, c, :, :, sl].transpose([1, 0, 2]))
            at = a_pool.tile([P, 2 * w], BF16, tag="a")
            nc.vector.tensor_mul(at[:], qt[:], kt[:])          # a = q*k
            nc.vector.tensor_add(at[:], at[:], gt[:])          # a += g
            nc.scalar.activation(
                at[:], at[:], mybir.ActivationFunctionType.Sigmoid
            )
            stage1[u] = (at, vt)

        def emit_stage2(u):
            bh, c, o, w = u
            sl = slice(o, o + w)
            at, vt = stage1.pop(u)
            nc.vector.tensor_mul(at[:], at[:], vt[:])          # u = s*v
            # yO = cumsum of (uE + uO) pairs: radix-2 scan over w columns,
            # written in place over uE (column t is read before written).
            if (c, o) != (0, 0):
                pt, pw = prev_y[bh]
                init = pt[:, pw - 1 : pw]
            else:
                init = 0.0
            nc.vector.tensor_tensor_scan(
                out=at[:, :w], data0=at[:, :w], data1=at[:, w:],
                initial=init,
                op0=mybir.AluOpType.add, op1=mybir.AluOpType.add,
            )
            nc.scalar.dma_start(out=y_d[bh, c, 1, :, sl], in_=at[:, :w])  # yO
            # yE = yO - uO, in place over uO.
            nc.vector.tensor_sub(at[:, w:], at[:, :w], at[:, w:])
            nc.scalar.dma_start(out=y_d[bh, c, 0, :, sl], in_=at[:, w:])  # yE
            prev_y[bh] = (at, w)

        for i, u in enumerate(units):
            emit_stage1(u, raw=(i < RAW_UNITS))
            if i >= LOOKAHEAD:
                emit_stage2(units[i - LOOKAHEAD])
        for u in units[-LOOKAHEAD:]:
            emit_stage2(u)

    nc.compile()  # bacc backend: wait legalization, reg alloc, nop fusion
    return nc


def kernel(q: np.ndarray, k: np.ndarray, v: np.ndarray, g: np.ndarray) -> np.ndarray:
    global _PROGRAM, LAST_RESULTS
    if _PROGRAM is None:
        _PROGRAM = _build_program()

    def prep(x, dt):
        # [B,H,N,D] f32 -> [BH, NCH, 2, D, N4]: per (b,h), time-major per
        # d lane, chunked then de-interleaved into even/odd steps.
        x = np.asarray(x, dtype=np.float32).reshape(BH, NCH, N4, 2, D)
        return np.ascontiguousarray(x.transpose(0, 1, 3, 4, 2)).astype(dt)

    qp, kp = prep(q, F8E3_NP), prep(k, F8E3_NP)
    vp, gp = prep(v, BF16_NP), prep(g, F8E3_NP)
    in_maps = []
    for i in range(N_CORES):
        s = slice(i * BH_PER_CORE, (i + 1) * BH_PER_CORE)
        in_maps.append({"q": qp[s], "k": kp[s], "v": vp[s], "g": gp[s]})

    LAST_RESULTS = run_bass_kernel_spmd(_PROGRAM, in_maps, core_ids=list(range(N_CORES)))
    y = np.concatenate([r["y"] for r in LAST_RESULTS.results], axis=0)
    # y: [BH, NCH, 2, D, N4], parity 0 = even positions, 1 = odd.
    # -> [BH, NCH, N4, 2(par), D] -> [B, H, N, D]
    y = y.transpose(0, 1, 4, 2, 3)
    return np.ascontiguousarray(y).astype(np.float32).reshape(B, H, N, D)
